# revision 15
# baseline (speedup 1.0000x reference)
"""Distributed Trainium2 Bass kernel for multi-head attention.

Reference computation (B=4, S=2048, D=1024, H=16 heads, HD=64):
    q = heads(Q @ Wq + bq + Q_lev)
    k = heads(K @ Wk + bk + K_lev)
    v = heads(V @ Wv + bv + V_lev)
    out = softmax(q k^T / sqrt(HD)) v  -> merge heads -> @ Wo + bo

Sharding: 8 cores = 4 batches x 2 head-halves (tensor parallel on the 16
heads: Wq/Wk/Wv split column-wise, Wo row-wise). Each core computes all
2048 queries for its 8 heads and a PARTIAL output [2048, 1024] = ctx_half
@ Wo_half (bf16); the host sums the two partials of each batch (+bo)
during the unshard. No duplicated projection compute and no on-device
collectives.

Device-side layout (feature-major / pre-transposed on the host):
  qT   [HH=512, S]  = Wq_half.T @ Q.T  (+ qlev = (bq + Q_lev).T half)
  kT   [HH, S]      = Wk_half.T @ K.T  (+ klev)
  vaug [tok, 8 heads, 65] = (V @ Wv_half + vlev) with a ones column
                            (row 64 of ctx = softmax denominator)
  scoresT[keys, q] = kT_h.T @ qT_h     (contract over HD=64)
  probsT = exp(scoresT / 8)            (no max subtraction: scores are
                                        N(0,~2) so exp stays < ~1e6)
  ctxT_aug[65, q] = vaug_h.T @ probsT
  ctxT = ctxT_aug[:64] * (1/denominator)  (fast reciprocal + K=2 ones
                                           matmul to broadcast across the
                                           64 head-dim partitions)
  out_partial[q, D] = ctxT.T @ Wo_half

Matmuls run in bf16 (f32 PSUM accumulation). The two K=64 scores matmuls
of a head pair run concurrently in PE row halves (tile_position derived
from base partitions 0/64) and write the two banks of one [128, 1024]
PSUM tile so a single wide ACT exp serves both heads.

Scheduling: ScalarE exp (~1 elem/cycle) and PE matmul streaming are
near-balanced (~285us vs ~275us), so the projections and output
projection are woven into the attention kc loop as PE "fillers" that
execute inside the exp-wait gaps, and the PE stream is software-
pipelined: scores(kc+1) issues before ctx(kc-2). The startup window is
DMA-bound: inputs/weights are fetched with a handful of large strided
DMAs (merged [128, chunk, cols] tiles) ordered by first use across the
three hardware DMA-issue queues (sync/scalar/gpsimd), so the first
scores run ~10us in and exp paces the rest. Only kT[0] n-block 0 and
qT[0] block 0 run before attention call 1; call 1's fillers carry the
rest of kT[0], the whole v projection (vaug[m] lands two kc steps before
ctx needs it) and kT[1]/qT[1]; later calls carry the next chunk's kT/qT
and the previous query block's output projection.
"""

import os
import sys

import numpy as np

for _p in ("/opt/trn_rl_repo", "/root/.axon_site/_ro/trn_rl_repo"):
    if os.path.isdir(_p) and _p not in sys.path:
        sys.path.insert(0, _p)

import ml_dtypes  # noqa: E402

B, S, D, H = 4, 2048, 1024, 16
HD = D // H  # 64
HH = D // 2  # 512 output-feature half per core
NH = H // 2  # 8 heads per core
N_CORES = 8
P = 128  # SBUF partitions
DC = D // P  # 8 chunks of the full (contraction) feature dim
MC = HH // P  # 4 chunks of my output-feature half
KC = S // P  # 16 key chunks
NB = 512  # matmul moving free-dim (one PSUM bank of f32)
NQB = S // NB  # 4 query blocks

_BUILD_CACHE = {}


def _build_nc():
    from concourse import bacc, mybir, tile

    f32 = mybir.dt.float32
    bf16 = mybir.dt.bfloat16
    Exp = mybir.ActivationFunctionType.Exp

    nc = bacc.Bacc("TRN2", target_bir_lowering=False, debug=False, num_devices=N_CORES)

    qt_d = nc.dram_tensor("qt", [D, S], bf16, kind="ExternalInput")
    qlev_d = nc.dram_tensor("qlev", [HH, S], bf16, kind="ExternalInput")
    kt_d = nc.dram_tensor("kt", [D, S], bf16, kind="ExternalInput")
    klev_d = nc.dram_tensor("klev", [HH, S], bf16, kind="ExternalInput")
    vt_d = nc.dram_tensor("vt", [D, S], bf16, kind="ExternalInput")
    vlev_d = nc.dram_tensor("vlev", [S, HH], bf16, kind="ExternalInput")
    wq_d = nc.dram_tensor("wq", [D, HH], bf16, kind="ExternalInput")
    wk_d = nc.dram_tensor("wk", [D, HH], bf16, kind="ExternalInput")
    wv_d = nc.dram_tensor("wv", [D, HH], bf16, kind="ExternalInput")
    wo_d = nc.dram_tensor("wo", [HH, D], bf16, kind="ExternalInput")
    ones2_d = nc.dram_tensor("ones2", [2, P], bf16, kind="ExternalInput")
    out_d = nc.dram_tensor("out", [S, D], bf16, kind="ExternalOutput")

    # [D, x] dram views as [P, DC, x] (partition-major for merged DMAs)
    qt_v = qt_d.rearrange("(i p) s -> p i s", p=P)
    kt_v = kt_d.rearrange("(i p) s -> p i s", p=P)
    vt_v = vt_d.rearrange("(i p) s -> p i s", p=P)
    wq_v = wq_d.rearrange("(i p) c -> p i c", p=P)
    wk_v = wk_d.rearrange("(i p) c -> p i c", p=P)
    wv_v = wv_d.rearrange("(i p) c -> p i c", p=P)
    wo_v = wo_d.rearrange("(i p) c -> p i c", p=P)

    with tile.TileContext(nc) as tc:
        with (
            tc.tile_pool(name="persist", bufs=1) as persist,
            tc.tile_pool(name="qinp", bufs=2) as qinp,
            tc.tile_pool(name="vinp", bufs=2) as vinp,
            tc.tile_pool(name="levk", bufs=2) as levkp,
            tc.tile_pool(name="lev", bufs=2) as levp,
            tc.tile_pool(name="probs", bufs=4) as prp,
            tc.tile_pool(name="norm", bufs=1) as nrm,
            tc.tile_pool(name="stgp", bufs=1) as stgp,
            tc.tile_pool(name="psum", bufs=1, space="PSUM") as psum,
        ):
            # Persistent intermediates (bf16).
            qT = [persist.tile([P, S], bf16, name=f"qT{i}", tag=f"qT{i}") for i in range(MC)]
            kT = [persist.tile([P, S], bf16, name=f"kT{i}", tag=f"kT{i}") for i in range(MC)]
            vaug = [
                persist.tile([P, NH, HD + 1], bf16, name=f"vaug{i}", tag=f"vaug{i}")
                for i in range(KC)
            ]
            ctxT = [persist.tile([P, S], bf16, name=f"ctxT{i}", tag=f"ctxT{i}") for i in range(MC)]
            # ones2[e, m] = 1 iff m // 64 == e: broadcasts the per-(head, q)
            # reciprocal across the 64 head-dim partitions via a K=2 matmul.
            ones2 = persist.tile([2, P], bf16, name="ones2", tag="ones2")
            # Merged weight/input tiles: one DMA each (DMA issue is ~600ns
            # per instruction on the issuing queue; the startup is gated on
            # instruction count as much as bytes).
            wk_sb = persist.tile([P, DC, HH], bf16, name="wk", tag="wk")
            wq_sb = persist.tile([P, DC, HH], bf16, name="wq", tag="wq")
            wv_sb = persist.tile([P, DC, HH], bf16, name="wv", tag="wv")
            wo_sb = persist.tile([P, MC, D], bf16, name="wo", tag="wo")
            kin = persist.tile([P, DC, S], bf16, name="kin", tag="kin")

            # ---- DMA issue order = priority order per queue ----
            # sync: kin n-block 0, qin(0), wv -> (later) vin, qin(qb), out
            nc.sync.dma_start(kin[:, :, 0:NB], kt_v[:, :, 0:NB])
            qin = {}

            def load_qin(n):
                t = qinp.tile([P, DC, NB], bf16, name="qin", tag="qin")
                nc.sync.dma_start(t[:], qt_v[:, :, n * NB : (n + 1) * NB])
                qin[n] = t

            load_qin(0)
            nc.sync.dma_start(wv_sb[:], wv_v[:])
            # scalar: wk, wq (pre-exp; scalar runs only ACTIVATE afterwards)
            nc.scalar.dma_start(wk_sb[:], wk_v[:])
            nc.scalar.dma_start(wq_sb[:], wq_v[:])
            nc.scalar.dma_start(ones2[:], ones2_d[:])
            # gpsimd: klev chunk 0 (prelude epilogue), kin n1, kin n2-3,
            # then the on-demand lev loads / wo / stash DMAs.
            klev0 = levkp.tile([P, S], bf16, name="klev0", tag="klev")
            nc.gpsimd.dma_start(klev0[:], klev_d[0:P, :])
            nc.gpsimd.dma_start(kin[:, :, NB : 2 * NB], kt_v[:, :, NB : 2 * NB])
            nc.gpsimd.dma_start(kin[:, :, 2 * NB : S], kt_v[:, :, 2 * NB : S])

            # ---------------- projection fillers -------------
            def kT_chunk_fillers(m, n0=0, klev_t=None):
                """kT[m] = Wk[:, m-chunk].T @ K.T: psum groups of 8
                accumulating matmuls + DVE epilogue each."""
                state = {}
                holder = {"lev": klev_t}
                fillers = []
                for n in range(n0, NQB):
                    for kc in range(DC):
                        def mmf(n=n, kc=kc):
                            if holder["lev"] is None:
                                t = levkp.tile([P, S], bf16, name="klev", tag="klev")
                                nc.gpsimd.dma_start(t[:], klev_d[m * P : (m + 1) * P, :])
                                holder["lev"] = t
                            if kc == 0:
                                state[n] = psum.tile(
                                    [P, NB], f32, name="psk", tag="ps_proj", bufs=2
                                )
                            nc.tensor.matmul(
                                state[n][:],
                                wk_sb[:, kc, m * P : (m + 1) * P],
                                kin[:, kc, n * NB : (n + 1) * NB],
                                start=(kc == 0),
                                stop=(kc == DC - 1),
                            )
                            if kc == DC - 1:
                                nc.vector.tensor_add(
                                    kT[m][:, n * NB : (n + 1) * NB],
                                    state[n][:],
                                    holder["lev"][:, n * NB : (n + 1) * NB],
                                )
                        fillers.append(mmf)
                return fillers

            def qT_group_fillers(m, n):
                state = {}
                fillers = []
                for kc in range(DC):
                    def mmf(kc=kc):
                        if kc == 0:
                            state[0] = psum.tile(
                                [P, NB], f32, name="psq", tag="ps_proj", bufs=2
                            )
                        nc.tensor.matmul(
                            state[0][:],
                            wq_sb[:, kc, m * P : (m + 1) * P],
                            qin[n][:, kc, :],
                            start=(kc == 0),
                            stop=(kc == DC - 1),
                        )
                        if kc == DC - 1:
                            lev = levp.tile([P, NB], bf16, name="levq", tag="lev")
                            nc.gpsimd.dma_start(
                                lev[:],
                                qlev_d[m * P : (m + 1) * P, n * NB : (n + 1) * NB],
                            )
                            nc.vector.tensor_add(
                                qT[m][:, n * NB : (n + 1) * NB], state[0][:], lev[:]
                            )
                    fillers.append(mmf)
                return fillers

            # v projection: vaug[m] (tokens m*128..) = V @ Wv_half + vlev,
            # head-strided with ones columns. 8 matmuls per chunk.
            vin = {}
            vstate = {}

            def v_chunk_fillers(m):
                c = m // 4
                fillers = []
                for kc in range(DC):
                    def mmf(kc=kc, m=m, c=c):
                        if kc == 0 and m % 4 == 0:
                            t = vinp.tile([P, DC, NB], bf16, name="vin", tag="vin")
                            nc.sync.dma_start(t[:], vt_v[:, :, c * NB : (c + 1) * NB])
                            vin[c] = t
                        if kc == 0:
                            vstate[0] = psum.tile(
                                [P, NB], f32, name="psv", tag="ps_proj", bufs=2
                            )
                        nc.tensor.matmul(
                            vstate[0][:],
                            vin[c][:, kc, (m % 4) * P : (m % 4 + 1) * P],
                            wv_sb[:, kc, :],
                            start=(kc == 0),
                            stop=(kc == DC - 1),
                        )
                        if kc == DC - 1:
                            lev = levp.tile([P, NB], bf16, name="levv", tag="lev")
                            nc.gpsimd.dma_start(
                                lev[:],
                                vlev_d[m * P : (m + 1) * P, 0:NB],
                            )
                            nc.vector.tensor_add(
                                vaug[m][:, :, 0:HD],
                                vstate[0][:].rearrange("p (h d) -> p h d", h=NH),
                                lev[:].rearrange("p (h d) -> p h d", h=NH),
                            )
                            nc.vector.memset(vaug[m][:, :, HD : HD + 1], 1.0)
                    fillers.append(mmf)
                return fillers

            def run_fillers(fillers, k):
                for _ in range(min(k, len(fillers))):
                    fillers.pop(0)()

            def emit_attention(qb, hp, fillers=None, per_kc=3):
                qs = slice(qb * NB, (qb + 1) * NB)
                fillers = fillers if fillers is not None else []
                cps = [
                    psum.tile([HD + 1, NB], f32, name=f"cps{e}", tag="ctxps", bufs=2)
                    for e in range(2)
                ]
                # software pipeline per kc: scores(kc); exp(kc); PE filler
                # work (projections/outproj) in the exp-wait gap; ctx(kc-2)
                # (lag 2 so ctx never waits on the just-issued exp; probs
                # bufs=4 covers the extra in-flight tile)
                LAG = 2
                prs = {}
                for kc in range(KC + LAG):
                    if kc < KC:
                        sps = psum.tile([P, 2 * NB], f32, name="sps", tag="sps", bufs=2)
                        for e in range(2):
                            rows = slice(e * HD, (e + 1) * HD)
                            # head pair packed in PE row halves
                            nc.tensor.matmul(
                                sps[:, e * NB : (e + 1) * NB],
                                kT[hp][rows, kc * P : (kc + 1) * P],
                                qT[hp][rows, qs],
                                start=True,
                                stop=True,
                            )
                        pr = prp.tile([P, 2 * NB], bf16, name="pr", tag="pr")
                        nc.scalar.activation(pr[:], sps[:], Exp, scale=1.0 / 8.0)
                        prs[kc] = pr
                        run_fillers(fillers, per_kc)
                    if kc >= LAG:
                        pkc = kc - LAG
                        ppr = prs.pop(pkc)
                        for e in range(2):
                            nc.tensor.matmul(
                                cps[e][:],
                                vaug[pkc][:, 2 * hp + e, :],
                                ppr[:, e * NB : (e + 1) * NB],
                                start=(pkc == 0),
                                stop=(pkc == KC - 1),
                            )
                run_fillers(fillers, len(fillers))
                sums2 = nrm.tile([2, NB], f32, name="sums2", tag="sums2", bufs=2)
                for e in range(2):
                    rows = slice(e * HD, (e + 1) * HD)
                    # Stash the denominator row: engines cannot write an
                    # arbitrary partition (bases limited to 0/32/64/96), so
                    # stage on partition 64 in SBUF then DMA into sums2[e].
                    stg = stgp.tile([HD + 1, NB], f32, name="stg", tag="stg")
                    nc.vector.tensor_copy(stg[HD : HD + 1, :], cps[e][HD : HD + 1, :])
                    nc.gpsimd.dma_start(sums2[e : e + 1, :], stg[HD : HD + 1, :])
                    # copy unnormalized ctx (normalized in place later)
                    nc.vector.tensor_copy(ctxT[hp][rows, qs], cps[e][0:HD, :])
                return (qb, hp, sums2)

            def emit_norm_finish(pend):
                # Normalize a head pair (deferred one call so the DVE queue
                # never waits on the sums2 DMA): 1/sums broadcast across the
                # 64 head-dim partitions via a K=2 matmul against ones2.
                qb, hp, sums2 = pend
                qs = slice(qb * NB, (qb + 1) * NB)
                recf2 = nrm.tile([2, NB], f32, name="recf2", tag="recf2")
                nc.vector.reciprocal_approx_fast(recf2[:], sums2[:])
                recb2 = nrm.tile([2, NB], bf16, name="recb2", tag="recb2")
                nc.vector.tensor_copy(recb2[:], recf2[:])
                bc = psum.tile([P, NB], f32, name="bc", tag="ps_proj", bufs=2)
                nc.tensor.matmul(bc[:], ones2[:], recb2[:], start=True, stop=True)
                nc.vector.tensor_mul(ctxT[hp][:, qs], ctxT[hp][:, qs], bc[:])

            def outproj_fillers(qg, n):
                state = {}
                fillers = []
                for dc in range(MC):
                    def mmf(dc=dc):
                        if dc == 0:
                            state[0] = psum.tile(
                                [P, NB], f32, name="pso", tag="ps_proj", bufs=2
                            )
                        nc.tensor.matmul(
                            state[0][:],
                            ctxT[dc][:, qg * P : (qg + 1) * P],
                            wo_sb[:, dc, n * NB : (n + 1) * NB],
                            start=(dc == 0),
                            stop=(dc == MC - 1),
                        )
                        if dc == MC - 1:
                            ot = nrm.tile([P, NB], bf16, name="ot", tag="otile", bufs=3)
                            nc.vector.tensor_copy(ot[:], state[0][:])
                            nc.sync.dma_start(
                                out_d[qg * P : (qg + 1) * P, n * NB : (n + 1) * NB],
                                ot[:],
                            )
                    fillers.append(mmf)
                return fillers

            # ---- interleaved schedule ----
            # Prelude: kT[0] n-block 0 and qT[0] qb0 only — PE gated by just
            # ~2.5MB of DMA.
            for f in kT_chunk_fillers(0, klev_t=klev0)[:DC]:
                f()
            for f in qT_group_fillers(0, 0):
                f()

            pend = None
            for qb in range(NQB):
                with nc.named_scope(f"qb{qb}"):
                    for hp in range(MC):
                        fillers = []
                        if qb == 0 and hp == 0:
                            # kT[0] n-blocks 1..3 first (scores kc=4n waits
                            # on n-block n), then the v proj (ctx(kc) needs
                            # vaug[kc] two steps ahead), then the NEXT
                            # call's kT/qT chunk.
                            fillers += kT_chunk_fillers(0, n0=1, klev_t=klev0)
                            for m in range(KC):
                                fillers += v_chunk_fillers(m)
                            fillers += kT_chunk_fillers(1)
                            fillers += qT_group_fillers(1, 0)
                        elif qb == 0:
                            if hp == 2:
                                # wo first used from qb1-hp1; gpsimd queue
                                # keeps it off the startup DMA window.
                                nc.gpsimd.dma_start(wo_sb[:], wo_v[:])
                            if hp < MC - 1:
                                fillers += kT_chunk_fillers(hp + 1)
                                fillers += qT_group_fillers(hp + 1, 0)
                        else:
                            if hp == 0:
                                for m in range(1, MC):
                                    fillers += qT_group_fillers(m, qb)
                            else:
                                # previous qb's outproj: 8 groups over 3 calls
                                og = [(4 * (qb - 1) + g, n) for g in range(4) for n in range(2)]
                                take = {1: og[0:3], 2: og[3:6], 3: og[6:8]}[hp]
                                for qg, n in take:
                                    fillers += outproj_fillers(qg, n)
                        if hp == MC - 1 and qb + 1 < NQB:
                            load_qin(qb + 1)
                            fillers += qT_group_fillers(0, qb + 1)
                        per_kc = (len(fillers) + KC - 1) // KC
                        nxt = emit_attention(qb, hp, fillers, per_kc=max(per_kc, 1))
                        if pend is not None:
                            emit_norm_finish(pend)
                        pend = nxt

            def emit_outproj_tail(qg, n, upto=MC):
                # dc 0..upto-1 into a fresh psum group; rest + epilogue later.
                # Rides the attention's (now idle) ctxps slots so the final
                # normalize's bcast matmul keeps a free ps_proj slot.
                ps = psum.tile([P, NB], f32, name="pso", tag="ctxps", bufs=2)
                for dc in range(upto):
                    nc.tensor.matmul(
                        ps[:],
                        ctxT[dc][:, qg * P : (qg + 1) * P],
                        wo_sb[:, dc, n * NB : (n + 1) * NB],
                        start=(dc == 0),
                        stop=(dc == MC - 1),
                    )
                def finish():
                    for dc in range(upto, MC):
                        nc.tensor.matmul(
                            ps[:],
                            ctxT[dc][:, qg * P : (qg + 1) * P],
                            wo_sb[:, dc, n * NB : (n + 1) * NB],
                            start=False,
                            stop=(dc == MC - 1),
                        )
                    ot = nrm.tile([P, NB], bf16, name="ot2", tag="otile", bufs=3)
                    nc.vector.tensor_copy(ot[:], ps[:])
                    nc.sync.dma_start(
                        out_d[qg * P : (qg + 1) * P, n * NB : (n + 1) * NB], ot[:]
                    )
                return finish

            with nc.named_scope("outproj_tail"):
                # last qb's 8 outproj groups; hp 0..2's ctxT chunks are
                # normalized already, so dc 0..2 partials can start before
                # the final (hp3) normalize enters the PE stream.
                tail = [(4 * (NQB - 1) + g, n) for g in range(4) for n in range(2)]
                f0 = emit_outproj_tail(*tail[0], upto=MC - 1)
                f1 = emit_outproj_tail(*tail[1], upto=MC - 1)
                emit_norm_finish(pend)
                f0()
                f1()
                for qg, n in tail[2:]:
                    f = emit_outproj_tail(qg, n)
                    f()

    nc.compile()
    return nc


def get_nc():
    if "nc" not in _BUILD_CACHE:
        _BUILD_CACHE["nc"] = _build_nc()
    return _BUILD_CACHE["nc"]


def make_in_maps(inputs):
    bf16 = ml_dtypes.bfloat16
    f32 = np.float32
    Q = np.asarray(inputs["Q"], f32)
    Q_lev = np.asarray(inputs["Q_lev"], f32)
    K = np.asarray(inputs["K"], f32)
    K_lev = np.asarray(inputs["K_lev"], f32)
    V = np.asarray(inputs["V"], f32)
    V_lev = np.asarray(inputs["V_lev"], f32)
    bq = np.asarray(inputs["bq"], f32)
    bk = np.asarray(inputs["bk"], f32)
    bv = np.asarray(inputs["bv"], f32)
    Wq = np.asarray(inputs["Wq"], f32)
    Wk = np.asarray(inputs["Wk"], f32)
    Wv = np.asarray(inputs["Wv"], f32)
    Wo = np.asarray(inputs["Wo"], f32)

    ones2 = np.zeros((2, P), f32)
    ones2[0, 0:HD] = 1.0
    ones2[1, HD : 2 * HD] = 1.0

    per_batch = []
    for b in range(B):
        per_batch.append(
            {
                "qt": np.ascontiguousarray(Q[b].T.astype(bf16)),
                "kt": np.ascontiguousarray(K[b].T.astype(bf16)),
                "vt": np.ascontiguousarray(V[b].T.astype(bf16)),
            }
        )
    qlevT = [np.ascontiguousarray((Q_lev[b] + bq).T).astype(bf16) for b in range(B)]
    klevT = [np.ascontiguousarray((K_lev[b] + bk).T).astype(bf16) for b in range(B)]
    vlev = [np.ascontiguousarray(V_lev[b] + bv).astype(bf16) for b in range(B)]

    in_maps = []
    for c in range(N_CORES):
        b, hh = divmod(c, 2)
        fs = slice(hh * HH, (hh + 1) * HH)
        in_maps.append(
            {
                **per_batch[b],
                "qlev": np.ascontiguousarray(qlevT[b][fs]),
                "klev": np.ascontiguousarray(klevT[b][fs]),
                "vlev": np.ascontiguousarray(vlev[b][:, fs]),
                "wq": np.ascontiguousarray(Wq[:, fs].astype(bf16)),
                "wk": np.ascontiguousarray(Wk[:, fs].astype(bf16)),
                "wv": np.ascontiguousarray(Wv[:, fs].astype(bf16)),
                "wo": np.ascontiguousarray(Wo[fs, :].astype(bf16)),
                "ones2": ones2.astype(bf16),
            }
        )
    return in_maps


def combine_outputs(results, inputs):
    bo = np.asarray(inputs["bo"], np.float32)
    out = np.empty((B, S, D), np.float32)
    for b in range(B):
        out[b] = (
            results[2 * b]["out"].astype(np.float32)
            + results[2 * b + 1]["out"].astype(np.float32)
            + bo
        )
    return out


def run_on_cores(inputs, trace=False):
    """Run the SPMD kernel; returns (full_output, BassKernelResults)."""
    from concourse.bass_utils import run_bass_kernel_spmd

    nc = get_nc()
    in_maps = make_in_maps(inputs)
    res = run_bass_kernel_spmd(nc, in_maps, core_ids=list(range(N_CORES)), trace=trace)
    return combine_outputs(res.results, inputs), res


def kernel(**inputs):
    out, _ = run_on_cores(inputs, trace=False)
    return out


if __name__ == "__main__":
    nc = get_nc()
    print("built + compiled OK")


# revision 17
# speedup vs baseline: 1.1900x; 1.1900x over previous
"""Distributed Trainium2 Bass kernel for multi-head attention.

Reference computation (B=4, S=2048, D=1024, H=16 heads, HD=64):
    q = heads(Q @ Wq + bq + Q_lev)
    k = heads(K @ Wk + bk + K_lev)
    v = heads(V @ Wv + bv + V_lev)
    out = softmax(q k^T / sqrt(HD)) v  -> merge heads -> @ Wo + bo

Sharding: 8 cores = 4 batches x 2 head-halves (tensor parallel on the 16
heads: Wq/Wk/Wv split column-wise, Wo row-wise). Each core computes all
2048 queries for its 8 heads and a PARTIAL output [2048, 1024] = ctx_half
@ Wo_half (bf16); the host sums the two partials of each batch (+bo)
during the unshard. No duplicated projection compute and no on-device
collectives.

Device-side layout (feature-major / pre-transposed on the host):
  qT   [HH=512, S]  = Wq_half.T @ Q.T  (+ qlev = (bq + Q_lev).T half)
  kT   [HH, S]      = Wk_half.T @ K.T  (+ klev)
  vaug [tok, 8 heads, 65] = (V @ Wv_half + vlev) with a ones column
                            (row 64 of ctx = softmax denominator)
  scoresT[keys, q] = kT_h.T @ qT_h     (contract over HD=64)
  probsT = exp(scoresT / 8)            (no max subtraction: scores are
                                        N(0,~2) so exp stays < ~1e6)
  ctxT_aug[65, q] = vaug_h.T @ probsT
  ctxT = ctxT_aug[:64] * (1/denominator)  (fast reciprocal + K=2 ones
                                           matmul to broadcast across the
                                           64 head-dim partitions)
  out_partial[q, D] = ctxT.T @ Wo_half

Matmuls run in bf16 (f32 PSUM accumulation). The two K=64 scores matmuls
of a head pair run concurrently in PE row halves (tile_position derived
from base partitions 0/64) and write the two banks of one [128, 1024]
PSUM tile so a single wide ACT exp serves both heads.

Scheduling: ScalarE exp (~1 elem/cycle) and PE matmul streaming are
near-balanced (~285us vs ~275us), so the projections and output
projection are woven into the attention kc loop as PE "fillers" that
execute inside the exp-wait gaps, and the PE stream is software-
pipelined: scores(kc+1) issues before ctx(kc-2). The startup window is
DMA-bound: inputs/weights are fetched with a handful of large strided
DMAs (merged [128, chunk, cols] tiles) ordered by first use across the
three hardware DMA-issue queues (sync/scalar/gpsimd), so the first
scores run ~10us in and exp paces the rest. Only kT[0] n-block 0 and
qT[0] block 0 run before attention call 1; call 1's fillers carry the
rest of kT[0], the whole v projection (vaug[m] lands two kc steps before
ctx needs it) and kT[1]/qT[1]; later calls carry the next chunk's kT/qT
and the previous query block's output projection.
"""

import os
import sys

import numpy as np

for _p in ("/opt/trn_rl_repo", "/root/.axon_site/_ro/trn_rl_repo"):
    if os.path.isdir(_p) and _p not in sys.path:
        sys.path.insert(0, _p)

import ml_dtypes  # noqa: E402

B, S, D, H = 4, 2048, 1024, 16
HD = D // H  # 64
HH = D // 2  # 512 output-feature half per core
NH = H // 2  # 8 heads per core
N_CORES = 8
P = 128  # SBUF partitions
DC = D // P  # 8 chunks of the full (contraction) feature dim
MC = HH // P  # 4 chunks of my output-feature half
KC = S // P  # 16 key chunks
NB = 512  # matmul moving free-dim (one PSUM bank of f32)
NQB = S // NB  # 4 query blocks

_BUILD_CACHE = {}


def _build_nc():
    from concourse import bacc, mybir, tile

    f32 = mybir.dt.float32
    bf16 = mybir.dt.bfloat16
    Exp = mybir.ActivationFunctionType.Exp

    nc = bacc.Bacc("TRN2", target_bir_lowering=False, debug=False, num_devices=N_CORES)

    qt_d = nc.dram_tensor("qt", [D, S], bf16, kind="ExternalInput")
    qlev_d = nc.dram_tensor("qlev", [HH, S], bf16, kind="ExternalInput")
    kt_d = nc.dram_tensor("kt", [D, S], bf16, kind="ExternalInput")
    klev_d = nc.dram_tensor("klev", [HH, S], bf16, kind="ExternalInput")
    vt_d = nc.dram_tensor("vt", [D, S], bf16, kind="ExternalInput")
    vlev_d = nc.dram_tensor("vlev", [S, HH], bf16, kind="ExternalInput")
    wq_d = nc.dram_tensor("wq", [D, HH], bf16, kind="ExternalInput")
    wk_d = nc.dram_tensor("wk", [D, HH], bf16, kind="ExternalInput")
    wv_d = nc.dram_tensor("wv", [D, HH], bf16, kind="ExternalInput")
    wo_d = nc.dram_tensor("wo", [HH, D], bf16, kind="ExternalInput")
    ones2_d = nc.dram_tensor("ones2", [2, P], bf16, kind="ExternalInput")
    out_d = nc.dram_tensor("out", [S, D], bf16, kind="ExternalOutput")

    # [D, x] dram views as [P, DC, x] (partition-major for merged DMAs)
    qt_v = qt_d.rearrange("(i p) s -> p i s", p=P)
    kt_v = kt_d.rearrange("(i p) s -> p i s", p=P)
    vt_v = vt_d.rearrange("(i p) s -> p i s", p=P)
    wq_v = wq_d.rearrange("(i p) c -> p i c", p=P)
    wk_v = wk_d.rearrange("(i p) c -> p i c", p=P)
    wv_v = wv_d.rearrange("(i p) c -> p i c", p=P)
    wo_v = wo_d.rearrange("(i p) c -> p i c", p=P)

    with tile.TileContext(nc) as tc:
        with (
            tc.tile_pool(name="persist", bufs=1) as persist,
            tc.tile_pool(name="qinp", bufs=2) as qinp,
            tc.tile_pool(name="vinp", bufs=2) as vinp,
            tc.tile_pool(name="levk", bufs=2) as levkp,
            tc.tile_pool(name="lev", bufs=2) as levp,
            tc.tile_pool(name="probs", bufs=4) as prp,
            tc.tile_pool(name="norm", bufs=1) as nrm,
            tc.tile_pool(name="stgp", bufs=1) as stgp,
            tc.tile_pool(name="psum", bufs=1, space="PSUM") as psum,
        ):
            # Persistent intermediates (bf16).
            qT = [persist.tile([P, S], bf16, name=f"qT{i}", tag=f"qT{i}") for i in range(MC)]
            kT = [persist.tile([P, S], bf16, name=f"kT{i}", tag=f"kT{i}") for i in range(MC)]
            vaug = [
                persist.tile([P, NH, HD + 1], bf16, name=f"vaug{i}", tag=f"vaug{i}")
                for i in range(KC)
            ]
            ctxT = [persist.tile([P, S], bf16, name=f"ctxT{i}", tag=f"ctxT{i}") for i in range(MC)]
            # ones2[e, m] = 1 iff m // 64 == e: broadcasts the per-(head, q)
            # reciprocal across the 64 head-dim partitions via a K=2 matmul.
            ones2 = persist.tile([2, P], bf16, name="ones2", tag="ones2")
            # Merged weight/input tiles: one DMA each (DMA issue is ~600ns
            # per instruction on the issuing queue; the startup is gated on
            # instruction count as much as bytes).
            wk_sb = persist.tile([P, DC, HH], bf16, name="wk", tag="wk")
            wq_sb = persist.tile([P, DC, HH], bf16, name="wq", tag="wq")
            wv_sb = persist.tile([P, DC, HH], bf16, name="wv", tag="wv")
            wo_sb = persist.tile([P, MC, D], bf16, name="wo", tag="wo")
            kin = persist.tile([P, DC, S], bf16, name="kin", tag="kin")

            # ---- DMA ordering ----
            # In-flight DMAs fair-share the 16 engines regardless of queue,
            # so priority only works by NOT having low-priority transfers in
            # flight: the whole startup-critical stream goes on the sync
            # queue in exact first-use order (a lone queue sustains
            # ~300GB/s); the small lev stream rides gpsimd; scalar stays
            # silent so exp is never behind a transfer.
            # sync: kin-n0, wk-c0, qin0, wq-c0, kin-n1, wv, then (woven into
            # call 1's fillers) vin-c0, kin-n2, vin-c1, kin-n3, vin-c2,
            # wk-rest, wq-rest, vin-c3.
            nc.sync.dma_start(kin[:, :, 0:NB], kt_v[:, :, 0:NB])
            nc.sync.dma_start(wk_sb[:, :, 0:P], wk_v[:, :, 0:P])
            qin = {}

            def load_qin(n):
                t = qinp.tile([P, DC, NB], bf16, name="qin", tag="qin")
                nc.sync.dma_start(t[:], qt_v[:, :, n * NB : (n + 1) * NB])
                qin[n] = t

            load_qin(0)
            nc.sync.dma_start(wq_sb[:, :, 0:P], wq_v[:, :, 0:P])
            nc.sync.dma_start(kin[:, :, NB : 2 * NB], kt_v[:, :, NB : 2 * NB])
            nc.sync.dma_start(wv_sb[:], wv_v[:])
            nc.scalar.dma_start(ones2[:], ones2_d[:])
            klev0 = levkp.tile([P, S], bf16, name="klev0", tag="klev")
            nc.gpsimd.dma_start(klev0[:], klev_d[0:P, :])

            # ---------------- projection fillers -------------
            def kT_chunk_fillers(m, n0=0, klev_t=None):
                """kT[m] = Wk[:, m-chunk].T @ K.T: psum groups of 8
                accumulating matmuls + DVE epilogue each."""
                state = {}
                holder = {"lev": klev_t}
                fillers = []
                for n in range(n0, NQB):
                    for kc in range(DC):
                        def mmf(n=n, kc=kc):
                            if holder["lev"] is None:
                                t = levkp.tile([P, S], bf16, name="klev", tag="klev")
                                nc.gpsimd.dma_start(t[:], klev_d[m * P : (m + 1) * P, :])
                                holder["lev"] = t
                            if kc == 0:
                                state[n] = psum.tile(
                                    [P, NB], f32, name="psk", tag="ps_proj", bufs=2
                                )
                            nc.tensor.matmul(
                                state[n][:],
                                wk_sb[:, kc, m * P : (m + 1) * P],
                                kin[:, kc, n * NB : (n + 1) * NB],
                                start=(kc == 0),
                                stop=(kc == DC - 1),
                            )
                            if kc == DC - 1:
                                nc.vector.tensor_add(
                                    kT[m][:, n * NB : (n + 1) * NB],
                                    state[n][:],
                                    holder["lev"][:, n * NB : (n + 1) * NB],
                                )
                        fillers.append(mmf)
                return fillers

            def qT_group_fillers(m, n):
                state = {}
                fillers = []
                for kc in range(DC):
                    def mmf(kc=kc):
                        if kc == 0:
                            state[0] = psum.tile(
                                [P, NB], f32, name="psq", tag="ps_proj", bufs=2
                            )
                        nc.tensor.matmul(
                            state[0][:],
                            wq_sb[:, kc, m * P : (m + 1) * P],
                            qin[n][:, kc, :],
                            start=(kc == 0),
                            stop=(kc == DC - 1),
                        )
                        if kc == DC - 1:
                            lev = levp.tile([P, NB], bf16, name="levq", tag="lev")
                            nc.gpsimd.dma_start(
                                lev[:],
                                qlev_d[m * P : (m + 1) * P, n * NB : (n + 1) * NB],
                            )
                            nc.vector.tensor_add(
                                qT[m][:, n * NB : (n + 1) * NB], state[0][:], lev[:]
                            )
                    fillers.append(mmf)
                return fillers

            # v projection: vaug[m] (tokens m*128..) = V @ Wv_half + vlev,
            # head-strided with ones columns. 8 matmuls per chunk.
            vin = {}
            vstate = {}

            def v_chunk_fillers(m):
                c = m // 4
                fillers = []
                for kc in range(DC):
                    def mmf(kc=kc, m=m, c=c):
                        if kc == 0 and m % 4 == 0:
                            t = vinp.tile([P, DC, NB], bf16, name="vin", tag="vin")
                            nc.sync.dma_start(t[:], vt_v[:, :, c * NB : (c + 1) * NB])
                            vin[c] = t
                        if kc == 0:
                            vstate[0] = psum.tile(
                                [P, NB], f32, name="psv", tag="ps_proj", bufs=2
                            )
                        nc.tensor.matmul(
                            vstate[0][:],
                            vin[c][:, kc, (m % 4) * P : (m % 4 + 1) * P],
                            wv_sb[:, kc, :],
                            start=(kc == 0),
                            stop=(kc == DC - 1),
                        )
                        if kc == DC - 1:
                            lev = levp.tile([P, NB], bf16, name="levv", tag="lev")
                            nc.gpsimd.dma_start(
                                lev[:],
                                vlev_d[m * P : (m + 1) * P, 0:NB],
                            )
                            nc.vector.tensor_add(
                                vaug[m][:, :, 0:HD],
                                vstate[0][:].rearrange("p (h d) -> p h d", h=NH),
                                lev[:].rearrange("p (h d) -> p h d", h=NH),
                            )
                            nc.vector.memset(vaug[m][:, :, HD : HD + 1], 1.0)
                    fillers.append(mmf)
                return fillers

            def run_fillers(fillers, k):
                for _ in range(min(k, len(fillers))):
                    fillers.pop(0)()

            def emit_attention(qb, hp, fillers=None, per_kc=3):
                qs = slice(qb * NB, (qb + 1) * NB)
                fillers = fillers if fillers is not None else []
                cps = [
                    psum.tile([HD + 1, NB], f32, name=f"cps{e}", tag="ctxps", bufs=2)
                    for e in range(2)
                ]
                # software pipeline per kc: scores(kc); exp(kc); PE filler
                # work (projections/outproj) in the exp-wait gap; ctx(kc-2)
                # (lag 2 so ctx never waits on the just-issued exp; probs
                # bufs=4 covers the extra in-flight tile)
                LAG = 2
                prs = {}
                for kc in range(KC + LAG):
                    if kc < KC:
                        sps = psum.tile([P, 2 * NB], f32, name="sps", tag="sps", bufs=2)
                        for e in range(2):
                            rows = slice(e * HD, (e + 1) * HD)
                            # head pair packed in PE row halves
                            nc.tensor.matmul(
                                sps[:, e * NB : (e + 1) * NB],
                                kT[hp][rows, kc * P : (kc + 1) * P],
                                qT[hp][rows, qs],
                                start=True,
                                stop=True,
                            )
                        pr = prp.tile([P, 2 * NB], bf16, name="pr", tag="pr")
                        nc.scalar.activation(pr[:], sps[:], Exp, scale=1.0 / 8.0)
                        prs[kc] = pr
                        run_fillers(fillers, per_kc)
                    if kc >= LAG:
                        pkc = kc - LAG
                        ppr = prs.pop(pkc)
                        for e in range(2):
                            nc.tensor.matmul(
                                cps[e][:],
                                vaug[pkc][:, 2 * hp + e, :],
                                ppr[:, e * NB : (e + 1) * NB],
                                start=(pkc == 0),
                                stop=(pkc == KC - 1),
                            )
                run_fillers(fillers, len(fillers))
                sums2 = nrm.tile([2, NB], f32, name="sums2", tag="sums2", bufs=2)
                for e in range(2):
                    rows = slice(e * HD, (e + 1) * HD)
                    # Stash the denominator row: engines cannot write an
                    # arbitrary partition (bases limited to 0/32/64/96), so
                    # stage on partition 64 in SBUF then DMA into sums2[e].
                    stg = stgp.tile([HD + 1, NB], f32, name="stg", tag="stg")
                    nc.vector.tensor_copy(stg[HD : HD + 1, :], cps[e][HD : HD + 1, :])
                    nc.gpsimd.dma_start(sums2[e : e + 1, :], stg[HD : HD + 1, :])
                    # copy unnormalized ctx (normalized in place later)
                    nc.vector.tensor_copy(ctxT[hp][rows, qs], cps[e][0:HD, :])
                return (qb, hp, sums2)

            def emit_norm_finish(pend):
                # Normalize a head pair (deferred one call so the DVE queue
                # never waits on the sums2 DMA): 1/sums broadcast across the
                # 64 head-dim partitions via a K=2 matmul against ones2.
                qb, hp, sums2 = pend
                qs = slice(qb * NB, (qb + 1) * NB)
                recf2 = nrm.tile([2, NB], f32, name="recf2", tag="recf2")
                nc.vector.reciprocal_approx_fast(recf2[:], sums2[:])
                recb2 = nrm.tile([2, NB], bf16, name="recb2", tag="recb2")
                nc.vector.tensor_copy(recb2[:], recf2[:])
                bc = psum.tile([P, NB], f32, name="bc", tag="ps_proj", bufs=2)
                nc.tensor.matmul(bc[:], ones2[:], recb2[:], start=True, stop=True)
                nc.vector.tensor_mul(ctxT[hp][:, qs], ctxT[hp][:, qs], bc[:])

            def outproj_fillers(qg, n):
                state = {}
                fillers = []
                for dc in range(MC):
                    def mmf(dc=dc):
                        if dc == 0:
                            state[0] = psum.tile(
                                [P, NB], f32, name="pso", tag="ps_proj", bufs=2
                            )
                        nc.tensor.matmul(
                            state[0][:],
                            ctxT[dc][:, qg * P : (qg + 1) * P],
                            wo_sb[:, dc, n * NB : (n + 1) * NB],
                            start=(dc == 0),
                            stop=(dc == MC - 1),
                        )
                        if dc == MC - 1:
                            ot = nrm.tile([P, NB], bf16, name="ot", tag="otile", bufs=3)
                            nc.vector.tensor_copy(ot[:], state[0][:])
                            nc.sync.dma_start(
                                out_d[qg * P : (qg + 1) * P, n * NB : (n + 1) * NB],
                                ot[:],
                            )
                    fillers.append(mmf)
                return fillers

            # ---- interleaved schedule ----
            # Prelude: kT[0] n-block 0 and qT[0] qb0 only — PE gated by just
            # ~2.5MB of DMA.
            for f in kT_chunk_fillers(0, klev_t=klev0)[:DC]:
                f()
            for f in qT_group_fillers(0, 0):
                f()

            pend = None
            for qb in range(NQB):
                with nc.named_scope(f"qb{qb}"):
                    for hp in range(MC):
                        fillers = []
                        if qb == 0 and hp == 0:
                            # Fillers interleaved to match both the compute
                            # deadlines (scores(4n) needs kT[0] n-block n;
                            # ctx(kc) needs vaug[kc] two steps ahead) and
                            # the sync-queue DMA stream order; demit
                            # closures push the later input DMAs at the
                            # right queue position.
                            kt0 = kT_chunk_fillers(0, n0=1, klev_t=klev0)
                            vch = [v_chunk_fillers(m) for m in range(KC)]

                            def demit(f):
                                fillers.append(f)

                            fillers += kt0[0:8]  # n1
                            fillers += vch[0]  # emits vin-c0
                            demit(lambda: nc.sync.dma_start(
                                kin[:, :, 2 * NB : 3 * NB], kt_v[:, :, 2 * NB : 3 * NB]))
                            fillers += vch[1] + vch[2] + vch[3]
                            fillers += kt0[8:16]  # n2
                            fillers += vch[4]  # emits vin-c1
                            demit(lambda: nc.sync.dma_start(
                                kin[:, :, 3 * NB : S], kt_v[:, :, 3 * NB : S]))
                            fillers += vch[5] + vch[6] + vch[7]
                            fillers += kt0[16:24]  # n3
                            fillers += vch[8]  # emits vin-c2
                            demit(lambda: nc.sync.dma_start(
                                wk_sb[:, :, P:HH], wk_v[:, :, P:HH]))
                            demit(lambda: nc.sync.dma_start(
                                wq_sb[:, :, P:HH], wq_v[:, :, P:HH]))
                            fillers += vch[9] + vch[10] + vch[11]
                            fillers += kT_chunk_fillers(1)
                            fillers += vch[12] + vch[13] + vch[14] + vch[15]
                            fillers += qT_group_fillers(1, 0)
                        elif qb == 0:
                            if hp == 2:
                                # wo first used from qb1-hp1; gpsimd queue
                                # keeps it off the startup DMA window.
                                nc.gpsimd.dma_start(wo_sb[:], wo_v[:])
                            if hp < MC - 1:
                                fillers += kT_chunk_fillers(hp + 1)
                                fillers += qT_group_fillers(hp + 1, 0)
                        else:
                            if hp == 0:
                                for m in range(1, MC):
                                    fillers += qT_group_fillers(m, qb)
                            else:
                                # previous qb's outproj: 8 groups over 3 calls
                                og = [(4 * (qb - 1) + g, n) for g in range(4) for n in range(2)]
                                take = {1: og[0:3], 2: og[3:6], 3: og[6:8]}[hp]
                                for qg, n in take:
                                    fillers += outproj_fillers(qg, n)
                        if hp == MC - 1 and qb + 1 < NQB:
                            load_qin(qb + 1)
                            fillers += qT_group_fillers(0, qb + 1)
                        per_kc = (len(fillers) + KC - 1) // KC
                        nxt = emit_attention(qb, hp, fillers, per_kc=max(per_kc, 1))
                        if pend is not None:
                            emit_norm_finish(pend)
                        pend = nxt

            def emit_outproj_tail(qg, n, upto=MC):
                # dc 0..upto-1 into a fresh psum group; rest + epilogue later.
                # Rides the attention's (now idle) ctxps slots so the final
                # normalize's bcast matmul keeps a free ps_proj slot.
                ps = psum.tile([P, NB], f32, name="pso", tag="ctxps", bufs=2)
                for dc in range(upto):
                    nc.tensor.matmul(
                        ps[:],
                        ctxT[dc][:, qg * P : (qg + 1) * P],
                        wo_sb[:, dc, n * NB : (n + 1) * NB],
                        start=(dc == 0),
                        stop=(dc == MC - 1),
                    )
                def finish():
                    for dc in range(upto, MC):
                        nc.tensor.matmul(
                            ps[:],
                            ctxT[dc][:, qg * P : (qg + 1) * P],
                            wo_sb[:, dc, n * NB : (n + 1) * NB],
                            start=False,
                            stop=(dc == MC - 1),
                        )
                    ot = nrm.tile([P, NB], bf16, name="ot2", tag="otile", bufs=3)
                    nc.vector.tensor_copy(ot[:], ps[:])
                    nc.sync.dma_start(
                        out_d[qg * P : (qg + 1) * P, n * NB : (n + 1) * NB], ot[:]
                    )
                return finish

            with nc.named_scope("outproj_tail"):
                # last qb's 8 outproj groups; hp 0..2's ctxT chunks are
                # normalized already, so dc 0..2 partials can start before
                # the final (hp3) normalize enters the PE stream.
                tail = [(4 * (NQB - 1) + g, n) for g in range(4) for n in range(2)]
                f0 = emit_outproj_tail(*tail[0], upto=MC - 1)
                f1 = emit_outproj_tail(*tail[1], upto=MC - 1)
                emit_norm_finish(pend)
                f0()
                f1()
                for qg, n in tail[2:]:
                    f = emit_outproj_tail(qg, n)
                    f()

    nc.compile()
    return nc


def get_nc():
    if "nc" not in _BUILD_CACHE:
        _BUILD_CACHE["nc"] = _build_nc()
    return _BUILD_CACHE["nc"]


def make_in_maps(inputs):
    bf16 = ml_dtypes.bfloat16
    f32 = np.float32
    Q = np.asarray(inputs["Q"], f32)
    Q_lev = np.asarray(inputs["Q_lev"], f32)
    K = np.asarray(inputs["K"], f32)
    K_lev = np.asarray(inputs["K_lev"], f32)
    V = np.asarray(inputs["V"], f32)
    V_lev = np.asarray(inputs["V_lev"], f32)
    bq = np.asarray(inputs["bq"], f32)
    bk = np.asarray(inputs["bk"], f32)
    bv = np.asarray(inputs["bv"], f32)
    Wq = np.asarray(inputs["Wq"], f32)
    Wk = np.asarray(inputs["Wk"], f32)
    Wv = np.asarray(inputs["Wv"], f32)
    Wo = np.asarray(inputs["Wo"], f32)

    ones2 = np.zeros((2, P), f32)
    ones2[0, 0:HD] = 1.0
    ones2[1, HD : 2 * HD] = 1.0

    per_batch = []
    for b in range(B):
        per_batch.append(
            {
                "qt": np.ascontiguousarray(Q[b].T.astype(bf16)),
                "kt": np.ascontiguousarray(K[b].T.astype(bf16)),
                "vt": np.ascontiguousarray(V[b].T.astype(bf16)),
            }
        )
    qlevT = [np.ascontiguousarray((Q_lev[b] + bq).T).astype(bf16) for b in range(B)]
    klevT = [np.ascontiguousarray((K_lev[b] + bk).T).astype(bf16) for b in range(B)]
    vlev = [np.ascontiguousarray(V_lev[b] + bv).astype(bf16) for b in range(B)]

    in_maps = []
    for c in range(N_CORES):
        b, hh = divmod(c, 2)
        fs = slice(hh * HH, (hh + 1) * HH)
        in_maps.append(
            {
                **per_batch[b],
                "qlev": np.ascontiguousarray(qlevT[b][fs]),
                "klev": np.ascontiguousarray(klevT[b][fs]),
                "vlev": np.ascontiguousarray(vlev[b][:, fs]),
                "wq": np.ascontiguousarray(Wq[:, fs].astype(bf16)),
                "wk": np.ascontiguousarray(Wk[:, fs].astype(bf16)),
                "wv": np.ascontiguousarray(Wv[:, fs].astype(bf16)),
                "wo": np.ascontiguousarray(Wo[fs, :].astype(bf16)),
                "ones2": ones2.astype(bf16),
            }
        )
    return in_maps


def combine_outputs(results, inputs):
    bo = np.asarray(inputs["bo"], np.float32)
    out = np.empty((B, S, D), np.float32)
    for b in range(B):
        out[b] = (
            results[2 * b]["out"].astype(np.float32)
            + results[2 * b + 1]["out"].astype(np.float32)
            + bo
        )
    return out


def run_on_cores(inputs, trace=False):
    """Run the SPMD kernel; returns (full_output, BassKernelResults)."""
    from concourse.bass_utils import run_bass_kernel_spmd

    nc = get_nc()
    in_maps = make_in_maps(inputs)
    res = run_bass_kernel_spmd(nc, in_maps, core_ids=list(range(N_CORES)), trace=trace)
    return combine_outputs(res.results, inputs), res


def kernel(**inputs):
    out, _ = run_on_cores(inputs, trace=False)
    return out


if __name__ == "__main__":
    nc = get_nc()
    print("built + compiled OK")
